# revision 18
# baseline (speedup 1.0000x reference)
"""Trainium2 Bass kernel for AdaptiveFeatureSelector (topk_masking).

Strategy (pure data parallel, 8 cores x 8192 rows):
 - All activations transposed on device: [features on partitions, rows on free].
 - Selector path (importance/gate nets -> combined) in fp16 hi/lo 3-term split
   matmuls (~f32 precision); reconstruction path in single fp16.
 - Exact per-row top-K=358 mask: PE-transpose combined to row-major, recenter
   by a host-calibrated affine-in-mean threshold estimate, then a 15-step
   binary search on fp16 residuals with per-partition tensor_scalar counts
   (accum_out). Mask built from the same fp16 residuals (consistent with the
   counts), PE-transposed back, applied to x, then recon MLP.
Host side: pre-transpose/split x, pre-split weights, calibrate the threshold
model on a 512-row sample; un-transpose the output. Device does all per-row
work.
"""

import sys

sys.path.insert(0, "/opt/trn_rl_repo")
import numpy as np

D = 512
H = 128
K = 358
B = 65536
NCORES = 8
R = B // NCORES          # rows per core
CHUNK = 512              # rows per chunk (one PSUM bank at f32)
NCHUNK = R // CHUNK      # 16
NCOL = R // 128          # 64 row-tiles per core
W_WIN = 0.025
ITERS = 15

_cache = {}


def _split16(a):
    hi = np.asarray(a, np.float16)
    lo = np.asarray(np.asarray(a, np.float32) - hi.astype(np.float32), np.float16)
    return hi, lo


def _sig(a):
    return 1.0 / (1.0 + np.exp(-a))


def _calibrate(x, W1, b1, W2, b2, W3, b3, Wg1, bg1, Wg2, bg2):
    xs = np.asarray(x[:512], np.float32)

    def mm3(a, Wm):
        ah, al = _split16(a)
        wh, wl = _split16(Wm)
        return (
            ah.astype(np.float32) @ wh.astype(np.float32)
            + ah.astype(np.float32) @ wl.astype(np.float32)
            + al.astype(np.float32) @ wh.astype(np.float32)
        )

    h = np.maximum(mm3(xs, W1) + b1, 0)
    h = np.maximum(mm3(h, W2) + b2, 0)
    imp = _sig(mm3(h, W3) + b3)
    g = np.maximum(mm3(xs, Wg1) + bg1, 0)
    gate = _sig(mm3(g, Wg2) + bg2)
    c = (imp * gate).astype(np.float32)
    mu = c.mean(1)
    thr = np.partition(c, D - K, axis=1)[:, D - K]
    A1, C1 = np.polyfit(mu, thr, 1)
    return float(A1), float(C1)


def _build_program():
    from concourse import bacc, mybir, tile

    f32 = mybir.dt.float32
    fp16 = mybir.dt.float16
    Act = mybir.ActivationFunctionType
    Alu = mybir.AluOpType

    nc = bacc.Bacc("TRN2", target_bir_lowering=False, debug=False,
                   num_devices=NCORES)

    def din(name, shape, dt=fp16):
        return nc.dram_tensor(name, shape, dt, kind="ExternalInput").ap()

    # DRAM layouts: weight K/M tile groups stored as [T, 128, 128] on host,
    # loaded into SBUF as [128, T, 128].
    xh_d = din("xh", [4, 128, R])
    xl_d = din("xl", [4, 128, R])
    w1h_d = din("w1h", [4, 128, 128]); w1l_d = din("w1l", [4, 128, 128])
    w2h_d = din("w2h", [128, 128]);    w2l_d = din("w2l", [128, 128])
    w3h_d = din("w3h", [4, 128, 128]); w3l_d = din("w3l", [4, 128, 128])
    wg1h_d = din("wg1h", [4, 128, 128]); wg1l_d = din("wg1l", [4, 128, 128])
    wg2h_d = din("wg2h", [4, 128, 128]); wg2l_d = din("wg2l", [4, 128, 128])
    wr1_d = din("wr1", [4, 128, 128])
    wr2_d = din("wr2", [128, K])
    b1_d = din("b1", [128, 1], f32)
    b2_d = din("b2", [128, 1], f32)
    b3_d = din("b3", [4, 128, 1], f32)
    bg1_d = din("bg1", [128, 1], f32)
    bg2_d = din("bg2", [4, 128, 1], f32)
    br1_d = din("br1", [128, 1], f32)
    br2_d = din("br2", [3, 128, 1], f32)   # padded 358 -> 384
    coef_d = din("coef", [128, 2], f32)
    idf32_d = din("idf32", [128, 128], f32)
    idf16_d = din("idf16", [128, 128], fp16)
    out_d = nc.dram_tensor("out", [K, R], f32, kind="ExternalOutput").ap()

    OSPLIT = [(0, 0, 128), (1, 128, 128), (2, 256, 102)]  # (tile, col0, width)

    with tile.TileContext(nc) as tc:
        with (
            tc.tile_pool(name="wts", bufs=1) as wts,
            tc.tile_pool(name="big", bufs=1) as big,
            tc.tile_pool(name="xls", bufs=4) as xls,
            tc.tile_pool(name="hbuf", bufs=3) as hbuf,
            tc.tile_pool(name="sgbuf", bufs=4) as sgbuf,
            tc.tile_pool(name="cbuf", bufs=4) as cbuf,
            tc.tile_pool(name="scr", bufs=2) as scr,
            tc.tile_pool(name="mbuf", bufs=4) as mbuf,
            tc.tile_pool(name="obuf", bufs=2) as obuf,
            tc.tile_pool(name="st", bufs=1) as st,
            tc.tile_pool(name="ps_h", bufs=2, space="PSUM") as ps_h,
            tc.tile_pool(name="ps_w", bufs=3, space="PSUM") as ps_w,
            tc.tile_pool(name="ps_t", bufs=2, space="PSUM") as ps_t,
        ):
            # ---- static loads (weights, biases, identities, coef) ----
            def ldt(dram, tiles, tag, dt=fp16):
                """[T,128,128] DRAM -> [128,T,128] SBUF."""
                t = wts.tile([128, tiles, 128], dt, tag=tag)
                nc.sync.dma_start(t, dram.rearrange("t p m -> p t m"))
                return t

            def ld2(dram, shape, tag, dt=f32):
                t = wts.tile(shape, dt, tag=tag)
                nc.sync.dma_start(t, dram)
                return t

            w1h = ldt(w1h_d, 4, "w1h"); w1l = ldt(w1l_d, 4, "w1l")
            w2h = ld2(w2h_d, [128, 128], "w2h", fp16)
            w2l = ld2(w2l_d, [128, 128], "w2l", fp16)
            w3h = ldt(w3h_d, 4, "w3h"); w3l = ldt(w3l_d, 4, "w3l")
            wg1h = ldt(wg1h_d, 4, "wg1h"); wg1l = ldt(wg1l_d, 4, "wg1l")
            wg2h = ldt(wg2h_d, 4, "wg2h"); wg2l = ldt(wg2l_d, 4, "wg2l")
            wr1 = ldt(wr1_d, 4, "wr1")
            wr2 = ld2(wr2_d, [128, K], "wr2", fp16)
            b1 = ld2(b1_d, [128, 1], "b1"); b2 = ld2(b2_d, [128, 1], "b2")
            b3 = wts.tile([128, 4, 1], f32, tag="b3")
            nc.sync.dma_start(b3, b3_d.rearrange("t p m -> p t m"))
            bg1 = ld2(bg1_d, [128, 1], "bg1")
            bg2 = wts.tile([128, 4, 1], f32, tag="bg2")
            nc.sync.dma_start(bg2, bg2_d.rearrange("t p m -> p t m"))
            br1 = ld2(br1_d, [128, 1], "br1")
            br2 = wts.tile([128, 3, 1], f32, tag="br2")
            nc.sync.dma_start(br2, br2_d.rearrange("t p m -> p t m"))
            coef = ld2(coef_d, [128, 2], "coef")
            idf32 = ld2(idf32_d, [128, 128], "idf32")
            idf16 = ld2(idf16_d, [128, 128], "idf16", fp16)

            # ---- persistent big buffers ----
            xh = big.tile([128, 4, R], fp16, tag="xh")      # resident x hi
            nc.sync.dma_start(xh, xh_d.rearrange("f p r -> p f r"))
            ebuf_g = []
            for g in range(4):
                tmp = big.tile([128, 16, 512], fp16, tag="ebuf%d" % g,
                               name="ebuf%d" % g)
                ebuf_g.append(tmp)
            mu = st.tile([128, NCOL], f32, tag="mu")
            t0 = st.tile([128, NCOL], f32, tag="t0")
            NG = 4
            GC = NCOL // NG
            lo_g, hi_g, mid_g, cnt_g, gek_g, gekn_g, nmid_g = [], [], [], [], [], [], []
            for g in range(NG):
                tmp = st.tile([128, GC], f32, tag="lo%d" % g, name="lo%d" % g)
                lo_g.append(tmp)
                tmp = st.tile([128, GC], f32, tag="hi%d" % g, name="hi%d" % g)
                hi_g.append(tmp)
                tmp = st.tile([128, GC], f32, tag="mid%d" % g, name="mid%d" % g)
                mid_g.append(tmp)
                tmp = st.tile([128, GC], f32, tag="cnt%d" % g, name="cnt%d" % g)
                cnt_g.append(tmp)
                tmp = st.tile([128, GC], mybir.dt.uint32, tag="gek%d" % g, name="gek%d" % g)
                gek_g.append(tmp)
                tmp = st.tile([128, GC], mybir.dt.uint32, tag="gekn%d" % g, name="gekn%d" % g)
                gekn_g.append(tmp)
                tmp = st.tile([128, GC], f32, tag="nmid%d" % g, name="nmid%d" % g)
                nmid_g.append(tmp)
            ones16 = st.tile([128, 512], fp16, tag="ones16")
            nc.vector.memset(ones16, 1.0)

            def split_evict(psum, bias):
                """relu(psum+bias) -> (hi, lo) fp16 pair."""
                hf = scr.tile([128, CHUNK], f32, tag="hf")
                nc.scalar.activation(hf, psum, Act.Relu, bias=bias)
                th = hbuf.tile([128, CHUNK], fp16, tag="hh")
                nc.vector.tensor_copy(th, hf)
                tl = hbuf.tile([128, CHUNK], fp16, tag="hl")
                nc.vector.scalar_tensor_tensor(
                    tl, th, -1.0, hf, op0=Alu.mult, op1=Alu.add)
                return th, tl

            def net3(stats, movs, psum):
                """3-term split matmuls accumulated into psum.

                stats: list of (stat_hi, stat_lo) [128,128] stationary APs
                movs: list of (mov_hi, mov_lo) [128,CHUNK] moving APs
                """
                nk = len(stats)
                idx = 0
                total = nk * 3
                for ki in range(nk):
                    sh_, sl_ = stats[ki]
                    mh, ml = movs[ki]
                    for sta, mov in ((sh_, mh), (sh_, ml), (sl_, mh)):
                        nc.tensor.matmul(psum, lhsT=sta, rhs=mov,
                                         start=(idx == 0), stop=(idx == total - 1))
                        idx += 1

            # =================== phase A: selector ===================
            def phase_a(ck):
                r0 = ck * CHUNK
                xhc = [xh[:, ft, r0:r0 + CHUNK] for ft in range(4)]
                xlc = []
                for ft in range(4):
                    t = xls.tile([128, CHUNK], fp16, tag="xl")
                    nc.sync.dma_start(t, xl_d[ft, :, r0:r0 + CHUNK])
                    xlc.append(t)
                xmov = [(xhc[ki], xlc[ki]) for ki in range(4)]

                # importance net
                p = ps_h.tile([128, CHUNK], f32, tag="h")
                net3([(w1h[:, ki, :], w1l[:, ki, :]) for ki in range(4)],
                     xmov, p)
                h1h, h1l = split_evict(p, b1)

                p = ps_h.tile([128, CHUNK], f32, tag="h")
                net3([(w2h, w2l)], [(h1h, h1l)], p)
                h2h, h2l = split_evict(p, b2)

                sa = []
                for mt in range(4):
                    pw = ps_w.tile([128, CHUNK], f32, tag="w")
                    net3([(w3h[:, mt, :], w3l[:, mt, :])], [(h2h, h2l)], pw)
                    t = sgbuf.tile([128, CHUNK], f32, tag="sa")
                    nc.scalar.activation(t, pw, Act.Sigmoid, bias=b3[:, mt, :])
                    sa.append(t)

                # gate net
                p = ps_h.tile([128, CHUNK], f32, tag="h")
                net3([(wg1h[:, ki, :], wg1l[:, ki, :]) for ki in range(4)],
                     xmov, p)
                g1h, g1l = split_evict(p, bg1)

                ct = []
                for mt in range(4):
                    pw = ps_w.tile([128, CHUNK], f32, tag="w")
                    net3([(wg2h[:, mt, :], wg2l[:, mt, :])], [(g1h, g1l)], pw)
                    t = sgbuf.tile([128, CHUNK], f32, tag="sg")
                    nc.scalar.activation(t, pw, Act.Sigmoid, bias=bg2[:, mt, :])
                    c = cbuf.tile([128, CHUNK], f32, tag="c")
                    nc.vector.tensor_mul(c, sa[mt], t)
                    ct.append(c)

                # transpose combined to row-major, accumulate row sums,
                # recenter into fp16 residuals
                for rt in range(4):
                    col = ck * 4 + rt
                    ptr = ps_t.tile([128, CHUNK], f32, tag="tr")
                    for mt in range(4):
                        nc.tensor.transpose(
                            ptr[:, mt * 128:(mt + 1) * 128],
                            ct[mt][:, rt * 128:(rt + 1) * 128], idf32)
                    crm = scr.tile([128, CHUNK], f32, tag="crm")
                    nc.scalar.activation(crm, ptr, Act.Identity,
                                         accum_out=mu[:, col:col + 1])
                    nc.vector.tensor_scalar(
                        t0[:, col:col + 1], mu[:, col:col + 1],
                        coef[:, 0:1], coef[:, 1:2],
                        op0=Alu.mult, op1=Alu.add)
                    nc.vector.tensor_scalar(
                        ebuf_g[col // GC][:, col % GC, :], crm,
                        t0[:, col:col + 1], None,
                        op0=Alu.subtract)

            # =================== phase B: binary search (per group) ===================
            def phase_b_init(g):
                nc.vector.memset(lo_g[g], -W_WIN)
                nc.vector.memset(hi_g[g], W_WIN)

            def phase_b_iter(g, it):
                lo, hi, mid = lo_g[g], hi_g[g], mid_g[g]
                cnt, gek, gekn, negmid = cnt_g[g], gek_g[g], gekn_g[g], nmid_g[g]
                nc.vector.tensor_add(mid, lo, hi)
                nc.vector.tensor_scalar_mul(mid, mid, 0.5)
                if (it + g) % 3 == 0:
                    nc.vector.tensor_scalar_mul(negmid, mid, -1.0)
                    for i in range(GC):
                        sc = scr.tile([128, 512], fp16, tag="csc%d" % g)
                        nc.scalar.activation(
                            sc, ebuf_g[g][:, i, :], Act.Sign,
                            bias=negmid[:, i:i + 1],
                            accum_out=cnt[:, i:i + 1])
                    thrv = float(2 * K - 512)
                else:
                    for i in range(GC):
                        sc = scr.tile([128, 512], fp16, tag="csc%d" % g)
                        nc.vector.scalar_tensor_tensor(
                            sc, ebuf_g[g][:, i, :], mid[:, i:i + 1], ones16,
                            op0=Alu.is_ge, op1=Alu.mult,
                            accum_out=cnt[:, i:i + 1])
                    thrv = float(K)
                nc.vector.tensor_scalar(gek, cnt, thrv, None, op0=Alu.is_ge)
                nc.vector.tensor_scalar(gekn, cnt, thrv, None, op0=Alu.is_lt)
                nc.vector.copy_predicated(lo, gek, mid)
                nc.vector.copy_predicated(hi, gekn, mid)

            # =================== phase C: mask + recon ===================
            def phase_c(ck, lo, c0):
                r0 = ck * CHUNK
                mrm = []
                for rt in range(4):
                    col = ck * 4 + rt
                    m = mbuf.tile([128, CHUNK], fp16, tag="mrm")
                    nc.vector.tensor_scalar(
                        m, ebuf_g[col // GC][:, col % GC, :],
                        lo[:, col - c0:col - c0 + 1], None,
                        op0=Alu.is_ge)
                    mrm.append(m)
                masked = []
                for ft in range(4):
                    pm = ps_t.tile([128, CHUNK], fp16, tag="tr")
                    for rt in range(4):
                        nc.tensor.transpose(
                            pm[:, rt * 128:(rt + 1) * 128],
                            mrm[rt][:, ft * 128:(ft + 1) * 128], idf16)
                    mk = mbuf.tile([128, CHUNK], fp16, tag="mk")
                    nc.vector.tensor_mul(mk, pm, xh[:, ft, r0:r0 + CHUNK])
                    masked.append(mk)

                p = ps_h.tile([128, CHUNK], f32, tag="h")
                for ft in range(4):
                    nc.tensor.matmul(p, lhsT=wr1[:, ft, :], rhs=masked[ft],
                                     start=(ft == 0), stop=(ft == 3))
                rh = hbuf.tile([128, CHUNK], fp16, tag="rh")
                nc.scalar.activation(rh, p, Act.Relu, bias=br1)

                for (ot, o0, ow) in OSPLIT:
                    po = ps_w.tile([128, CHUNK], f32, tag="w")
                    nc.tensor.matmul(po[0:ow, :], lhsT=wr2[:, o0:o0 + ow],
                                     rhs=rh, start=True, stop=True)
                    of = obuf.tile([128, CHUNK], f32, tag="of")
                    nc.scalar.activation(of[0:ow, :], po[0:ow, :], Act.Identity,
                                         bias=br2[0:ow, ot, :])
                    nc.sync.dma_start(out_d[o0:o0 + ow, r0:r0 + CHUNK],
                                      of[0:ow, :])

            CPG = NCHUNK // NG
            for ck in range(NCHUNK):
                phase_a(ck)
            for g in range(NG):
                phase_b_init(g)
            for it in range(ITERS):
                for g in range(NG):
                    phase_b_iter(g, it)
            for g in range(NG):
                for ck in range(g * CPG, (g + 1) * CPG):
                    phase_c(ck, lo_g[g], g * GC)

    nc.compile()
    return nc


def kernel(**inputs):
    from concourse.bass_utils import run_bass_kernel_spmd

    x = np.asarray(inputs["x"], np.float32)
    names = ["W1", "b1", "W2", "b2", "W3", "b3", "Wg1", "bg1", "Wg2", "bg2",
             "Wr1", "br1", "Wr2", "br2"]
    W1, b1, W2, b2, W3, b3, Wg1, bg1, Wg2, bg2, Wr1, br1, Wr2, br2 = (
        np.asarray(inputs[n], np.float32) for n in names)

    A1, C1 = _calibrate(x, W1, b1, W2, b2, W3, b3, Wg1, bg1, Wg2, bg2)

    # weight prep (shared by all cores)
    def ksplit(Wm):  # [512,128] -> hi/lo [4,128,128] K tiles
        h, l = _split16(Wm)
        return (np.ascontiguousarray(h.reshape(4, 128, 128)),
                np.ascontiguousarray(l.reshape(4, 128, 128)))

    def msplit(Wm):  # [128,512] -> hi/lo [4,128,128] M tiles
        h, l = _split16(Wm)
        return (np.ascontiguousarray(h.reshape(128, 4, 128).transpose(1, 0, 2)),
                np.ascontiguousarray(l.reshape(128, 4, 128).transpose(1, 0, 2)))

    w1h, w1l = ksplit(W1)
    w2h, w2l = _split16(W2)
    w3h, w3l = msplit(W3)
    wg1h, wg1l = ksplit(Wg1)
    wg2h, wg2l = msplit(Wg2)
    wr1 = np.ascontiguousarray(
        np.asarray(Wr1, np.float16).reshape(4, 128, 128))
    wr2 = np.ascontiguousarray(np.asarray(Wr2, np.float16))
    coef = np.zeros((128, 2), np.float32)
    coef[:, 0] = A1 / 512.0
    coef[:, 1] = C1
    br2pad = np.zeros((3, 128, 1), np.float32)
    br2pad.reshape(-1)[:K] = br2
    ident = np.eye(128)
    shared = dict(
        w1h=w1h, w1l=w1l,
        w2h=np.ascontiguousarray(w2h), w2l=np.ascontiguousarray(w2l),
        w3h=w3h, w3l=w3l,
        wg1h=wg1h, wg1l=wg1l, wg2h=wg2h, wg2l=wg2l,
        wr1=wr1, wr2=wr2,
        b1=b1.reshape(128, 1), b2=b2.reshape(128, 1),
        b3=np.ascontiguousarray(b3.reshape(4, 128, 1)),
        bg1=bg1.reshape(128, 1),
        bg2=np.ascontiguousarray(bg2.reshape(4, 128, 1)),
        br1=br1.reshape(128, 1), br2=br2pad,
        coef=coef,
        idf32=ident.astype(np.float32),
        idf16=ident.astype(np.float16),
    )

    in_maps = []
    for i in range(NCORES):
        xs = x[i * R:(i + 1) * R]                 # [R, 512]
        xT = np.ascontiguousarray(xs.T)           # [512, R]
        xTh = np.asarray(xT, np.float16)
        xTl = np.asarray(xT - xTh.astype(np.float32), np.float16)
        m = dict(shared)
        m["xh"] = np.ascontiguousarray(xTh.reshape(4, 128, R))
        m["xl"] = np.ascontiguousarray(xTl.reshape(4, 128, R))
        in_maps.append(m)

    if "nc" not in _cache:
        _cache["nc"] = _build_program()
    nc = _cache["nc"]
    _cache["in_maps"] = in_maps

    res = run_bass_kernel_spmd(nc, in_maps, list(range(NCORES)))
    out = np.concatenate(
        [np.ascontiguousarray(res.results[i]["out"].T) for i in range(NCORES)],
        axis=0)
    return out.astype(np.float32)


if __name__ == "__main__":
    rng = np.random.default_rng(0)
    fake = {"x": rng.standard_normal((B, D), dtype=np.float32)}
    s = lambda f: 1.0 / np.sqrt(f)
    for nm, sh, fan in [("W1", (D, H), D), ("W2", (H, H), H), ("W3", (H, D), H),
                        ("Wg1", (D, H), D), ("Wg2", (H, D), H),
                        ("Wr1", (D, H), D), ("Wr2", (H, K), H)]:
        fake[nm] = rng.uniform(-s(fan), s(fan), sh).astype(np.float32)
    for nm, sh in [("b1", H), ("b2", H), ("b3", D), ("bg1", H), ("bg2", D),
                   ("br1", H), ("br2", K)]:
        fake[nm] = np.zeros(sh, np.float32)
    out = kernel(**fake)
    print("out", out.shape, out.dtype, float(np.abs(out).max()))


# revision 19
# speedup vs baseline: 1.1462x; 1.1462x over previous
"""Trainium2 Bass kernel for AdaptiveFeatureSelector (topk_masking).

Strategy (pure data parallel, 8 cores x 8192 rows):
 - All activations transposed on device: [features on partitions, rows on free].
 - Selector path (importance/gate nets -> combined) in fp16 hi/lo 3-term split
   matmuls (~f32 precision); reconstruction path in single fp16.
 - Exact per-row top-K=358 mask: PE-transpose combined to row-major, recenter
   by a host-calibrated affine-in-mean threshold estimate, then a 15-step
   binary search on fp16 residuals with per-partition tensor_scalar counts
   (accum_out). Mask built from the same fp16 residuals (consistent with the
   counts), PE-transposed back, applied to x, then recon MLP.
Host side: pre-transpose/split x, pre-split weights, calibrate the threshold
model on a 512-row sample; un-transpose the output. Device does all per-row
work.
"""

import sys

sys.path.insert(0, "/opt/trn_rl_repo")
import numpy as np

D = 512
H = 128
K = 358
B = 65536
NCORES = 8
R = B // NCORES          # rows per core
CHUNK = 512              # rows per chunk (one PSUM bank at f32)
NCHUNK = R // CHUNK      # 16
NCOL = R // 128          # 64 row-tiles per core
W_WIN = 0.025
ITERS = 14

_cache = {}


def _split16(a):
    hi = np.asarray(a, np.float16)
    lo = np.asarray(np.asarray(a, np.float32) - hi.astype(np.float32), np.float16)
    return hi, lo


def _sig(a):
    return 1.0 / (1.0 + np.exp(-a))


def _calibrate(x, W1, b1, W2, b2, W3, b3, Wg1, bg1, Wg2, bg2):
    xs = np.asarray(x[:512], np.float32)

    def mm3(a, Wm):
        ah, al = _split16(a)
        wh, wl = _split16(Wm)
        return (
            ah.astype(np.float32) @ wh.astype(np.float32)
            + ah.astype(np.float32) @ wl.astype(np.float32)
            + al.astype(np.float32) @ wh.astype(np.float32)
        )

    h = np.maximum(mm3(xs, W1) + b1, 0)
    h = np.maximum(mm3(h, W2) + b2, 0)
    imp = _sig(mm3(h, W3) + b3)
    g = np.maximum(mm3(xs, Wg1) + bg1, 0)
    gate = _sig(mm3(g, Wg2) + bg2)
    c = (imp * gate).astype(np.float32)
    mu = c.mean(1)
    thr = np.partition(c, D - K, axis=1)[:, D - K]
    A1, C1 = np.polyfit(mu, thr, 1)
    return float(A1), float(C1)


def _build_program():
    from concourse import bacc, mybir, tile

    f32 = mybir.dt.float32
    fp16 = mybir.dt.float16
    Act = mybir.ActivationFunctionType
    Alu = mybir.AluOpType

    nc = bacc.Bacc("TRN2", target_bir_lowering=False, debug=False,
                   num_devices=NCORES)

    def din(name, shape, dt=fp16):
        return nc.dram_tensor(name, shape, dt, kind="ExternalInput").ap()

    # DRAM layouts: weight K/M tile groups stored as [T, 128, 128] on host,
    # loaded into SBUF as [128, T, 128].
    xh_d = din("xh", [4, 128, R])
    xl_d = din("xl", [4, 128, R])
    w1h_d = din("w1h", [4, 128, 128]); w1l_d = din("w1l", [4, 128, 128])
    w2h_d = din("w2h", [128, 128]);    w2l_d = din("w2l", [128, 128])
    w3h_d = din("w3h", [4, 128, 128]); w3l_d = din("w3l", [4, 128, 128])
    wg1h_d = din("wg1h", [4, 128, 128]); wg1l_d = din("wg1l", [4, 128, 128])
    wg2h_d = din("wg2h", [4, 128, 128]); wg2l_d = din("wg2l", [4, 128, 128])
    wr1_d = din("wr1", [4, 128, 128])
    wr2_d = din("wr2", [128, K])
    b1_d = din("b1", [128, 1], f32)
    b2_d = din("b2", [128, 1], f32)
    b3_d = din("b3", [4, 128, 1], f32)
    bg1_d = din("bg1", [128, 1], f32)
    bg2_d = din("bg2", [4, 128, 1], f32)
    br1_d = din("br1", [128, 1], f32)
    br2_d = din("br2", [3, 128, 1], f32)   # padded 358 -> 384
    coef_d = din("coef", [128, 2], f32)
    idf32_d = din("idf32", [128, 128], f32)
    idf16_d = din("idf16", [128, 128], fp16)
    out_d = nc.dram_tensor("out", [K, R], f32, kind="ExternalOutput").ap()

    OSPLIT = [(0, 0, 128), (1, 128, 128), (2, 256, 102)]  # (tile, col0, width)

    with tile.TileContext(nc) as tc:
        with (
            tc.tile_pool(name="wts", bufs=1) as wts,
            tc.tile_pool(name="big", bufs=1) as big,
            tc.tile_pool(name="xls", bufs=4) as xls,
            tc.tile_pool(name="hbuf", bufs=3) as hbuf,
            tc.tile_pool(name="sgbuf", bufs=4) as sgbuf,
            tc.tile_pool(name="cbuf", bufs=4) as cbuf,
            tc.tile_pool(name="scr", bufs=2) as scr,
            tc.tile_pool(name="mbuf", bufs=4) as mbuf,
            tc.tile_pool(name="obuf", bufs=2) as obuf,
            tc.tile_pool(name="st", bufs=1) as st,
            tc.tile_pool(name="ps_h", bufs=2, space="PSUM") as ps_h,
            tc.tile_pool(name="ps_w", bufs=3, space="PSUM") as ps_w,
            tc.tile_pool(name="ps_t", bufs=2, space="PSUM") as ps_t,
        ):
            # ---- static loads (weights, biases, identities, coef) ----
            def ldt(dram, tiles, tag, dt=fp16):
                """[T,128,128] DRAM -> [128,T,128] SBUF."""
                t = wts.tile([128, tiles, 128], dt, tag=tag)
                nc.sync.dma_start(t, dram.rearrange("t p m -> p t m"))
                return t

            def ld2(dram, shape, tag, dt=f32):
                t = wts.tile(shape, dt, tag=tag)
                nc.sync.dma_start(t, dram)
                return t

            w1h = ldt(w1h_d, 4, "w1h"); w1l = ldt(w1l_d, 4, "w1l")
            w2h = ld2(w2h_d, [128, 128], "w2h", fp16)
            w2l = ld2(w2l_d, [128, 128], "w2l", fp16)
            w3h = ldt(w3h_d, 4, "w3h"); w3l = ldt(w3l_d, 4, "w3l")
            wg1h = ldt(wg1h_d, 4, "wg1h"); wg1l = ldt(wg1l_d, 4, "wg1l")
            wg2h = ldt(wg2h_d, 4, "wg2h"); wg2l = ldt(wg2l_d, 4, "wg2l")
            wr1 = ldt(wr1_d, 4, "wr1")
            wr2 = ld2(wr2_d, [128, K], "wr2", fp16)
            b1 = ld2(b1_d, [128, 1], "b1"); b2 = ld2(b2_d, [128, 1], "b2")
            b3 = wts.tile([128, 4, 1], f32, tag="b3")
            nc.sync.dma_start(b3, b3_d.rearrange("t p m -> p t m"))
            bg1 = ld2(bg1_d, [128, 1], "bg1")
            bg2 = wts.tile([128, 4, 1], f32, tag="bg2")
            nc.sync.dma_start(bg2, bg2_d.rearrange("t p m -> p t m"))
            br1 = ld2(br1_d, [128, 1], "br1")
            br2 = wts.tile([128, 3, 1], f32, tag="br2")
            nc.sync.dma_start(br2, br2_d.rearrange("t p m -> p t m"))
            coef = ld2(coef_d, [128, 2], "coef")
            idf32 = ld2(idf32_d, [128, 128], "idf32")
            idf16 = ld2(idf16_d, [128, 128], "idf16", fp16)

            # ---- persistent big buffers ----
            xh = big.tile([128, 4, R], fp16, tag="xh")      # resident x hi
            nc.sync.dma_start(xh, xh_d.rearrange("f p r -> p f r"))
            ebuf_g = []
            for g in range(4):
                tmp = big.tile([128, 16, 512], fp16, tag="ebuf%d" % g,
                               name="ebuf%d" % g)
                ebuf_g.append(tmp)
            mu = st.tile([128, NCOL], f32, tag="mu")
            t0 = st.tile([128, NCOL], f32, tag="t0")
            NG = 4
            GC = NCOL // NG
            lo_g, hi_g, mid_g, cnt_g, gek_g, gekn_g, nmid_g = [], [], [], [], [], [], []
            for g in range(NG):
                tmp = st.tile([128, GC], f32, tag="lo%d" % g, name="lo%d" % g)
                lo_g.append(tmp)
                tmp = st.tile([128, GC], f32, tag="hi%d" % g, name="hi%d" % g)
                hi_g.append(tmp)
                tmp = st.tile([128, GC], f32, tag="mid%d" % g, name="mid%d" % g)
                mid_g.append(tmp)
                tmp = st.tile([128, GC], f32, tag="cnt%d" % g, name="cnt%d" % g)
                cnt_g.append(tmp)
                tmp = st.tile([128, GC], mybir.dt.uint32, tag="gek%d" % g, name="gek%d" % g)
                gek_g.append(tmp)
                tmp = st.tile([128, GC], mybir.dt.uint32, tag="gekn%d" % g, name="gekn%d" % g)
                gekn_g.append(tmp)
                tmp = st.tile([128, GC], f32, tag="nmid%d" % g, name="nmid%d" % g)
                nmid_g.append(tmp)
            ones16 = st.tile([128, 512], fp16, tag="ones16")
            nc.vector.memset(ones16, 1.0)

            def split_evict(psum, bias):
                """relu(psum+bias) -> (hi, lo) fp16 pair."""
                hf = scr.tile([128, CHUNK], f32, tag="hf")
                nc.scalar.activation(hf, psum, Act.Relu, bias=bias)
                th = hbuf.tile([128, CHUNK], fp16, tag="hh")
                nc.vector.tensor_copy(th, hf)
                tl = hbuf.tile([128, CHUNK], fp16, tag="hl")
                nc.vector.scalar_tensor_tensor(
                    tl, th, -1.0, hf, op0=Alu.mult, op1=Alu.add)
                return th, tl

            def net3(stats, movs, psum):
                """3-term split matmuls accumulated into psum.

                stats: list of (stat_hi, stat_lo) [128,128] stationary APs
                movs: list of (mov_hi, mov_lo) [128,CHUNK] moving APs
                """
                nk = len(stats)
                idx = 0
                total = nk * 3
                for ki in range(nk):
                    sh_, sl_ = stats[ki]
                    mh, ml = movs[ki]
                    for sta, mov in ((sh_, mh), (sh_, ml), (sl_, mh)):
                        nc.tensor.matmul(psum, lhsT=sta, rhs=mov,
                                         start=(idx == 0), stop=(idx == total - 1))
                        idx += 1

            # =================== phase A: selector ===================
            def phase_a(ck):
                r0 = ck * CHUNK
                xhc = [xh[:, ft, r0:r0 + CHUNK] for ft in range(4)]
                xlc = []
                for ft in range(4):
                    t = xls.tile([128, CHUNK], fp16, tag="xl")
                    nc.sync.dma_start(t, xl_d[ft, :, r0:r0 + CHUNK])
                    xlc.append(t)
                xmov = [(xhc[ki], xlc[ki]) for ki in range(4)]

                # importance net
                p = ps_h.tile([128, CHUNK], f32, tag="h")
                net3([(w1h[:, ki, :], w1l[:, ki, :]) for ki in range(4)],
                     xmov, p)
                h1h, h1l = split_evict(p, b1)

                p = ps_h.tile([128, CHUNK], f32, tag="h")
                net3([(w2h, w2l)], [(h1h, h1l)], p)
                h2h, h2l = split_evict(p, b2)

                sa = []
                for mt in range(4):
                    pw = ps_w.tile([128, CHUNK], f32, tag="w")
                    net3([(w3h[:, mt, :], w3l[:, mt, :])], [(h2h, h2l)], pw)
                    t = sgbuf.tile([128, CHUNK], f32, tag="sa")
                    nc.scalar.activation(t, pw, Act.Sigmoid, bias=b3[:, mt, :])
                    sa.append(t)

                # gate net
                p = ps_h.tile([128, CHUNK], f32, tag="h")
                net3([(wg1h[:, ki, :], wg1l[:, ki, :]) for ki in range(4)],
                     xmov, p)
                g1h, g1l = split_evict(p, bg1)

                ct = []
                for mt in range(4):
                    pw = ps_w.tile([128, CHUNK], f32, tag="w")
                    net3([(wg2h[:, mt, :], wg2l[:, mt, :])], [(g1h, g1l)], pw)
                    t = sgbuf.tile([128, CHUNK], f32, tag="sg")
                    nc.scalar.activation(t, pw, Act.Sigmoid, bias=bg2[:, mt, :])
                    c = cbuf.tile([128, CHUNK], f32, tag="c")
                    nc.vector.tensor_mul(c, sa[mt], t)
                    ct.append(c)

                # transpose combined to row-major, accumulate row sums,
                # recenter into fp16 residuals
                for rt in range(4):
                    col = ck * 4 + rt
                    ptr = ps_t.tile([128, CHUNK], f32, tag="tr")
                    for mt in range(4):
                        nc.tensor.transpose(
                            ptr[:, mt * 128:(mt + 1) * 128],
                            ct[mt][:, rt * 128:(rt + 1) * 128], idf32)
                    crm = scr.tile([128, CHUNK], f32, tag="crm")
                    nc.scalar.activation(crm, ptr, Act.Identity,
                                         accum_out=mu[:, col:col + 1])
                    nc.vector.tensor_scalar(
                        t0[:, col:col + 1], mu[:, col:col + 1],
                        coef[:, 0:1], coef[:, 1:2],
                        op0=Alu.mult, op1=Alu.add)
                    nc.vector.tensor_scalar(
                        ebuf_g[col // GC][:, col % GC, :], crm,
                        t0[:, col:col + 1], None,
                        op0=Alu.subtract)

            # =================== phase B: binary search (per group) ===================
            def phase_b_init(g):
                nc.vector.memset(lo_g[g], -W_WIN)
                nc.vector.memset(hi_g[g], W_WIN)

            def phase_b_iter(g, it):
                lo, hi, mid = lo_g[g], hi_g[g], mid_g[g]
                cnt, gek, gekn, negmid = cnt_g[g], gek_g[g], gekn_g[g], nmid_g[g]
                nc.vector.tensor_add(mid, lo, hi)
                nc.vector.tensor_scalar_mul(mid, mid, 0.5)
                if (it + g) % 2 == 0:
                    nc.vector.tensor_scalar_mul(negmid, mid, -1.0)
                    for i in range(GC):
                        sc = scr.tile([128, 512], fp16, tag="csc%d" % g)
                        nc.scalar.activation(
                            sc, ebuf_g[g][:, i, :], Act.Sign,
                            bias=negmid[:, i:i + 1],
                            accum_out=cnt[:, i:i + 1])
                    thrv = float(2 * K - 512)
                else:
                    for i in range(GC):
                        sc = scr.tile([128, 512], fp16, tag="csc%d" % g)
                        nc.vector.scalar_tensor_tensor(
                            sc, ebuf_g[g][:, i, :], mid[:, i:i + 1], ones16,
                            op0=Alu.is_ge, op1=Alu.mult,
                            accum_out=cnt[:, i:i + 1])
                    thrv = float(K)
                nc.vector.tensor_scalar(gek, cnt, thrv, None, op0=Alu.is_ge)
                nc.vector.tensor_scalar(gekn, cnt, thrv, None, op0=Alu.is_lt)
                nc.vector.copy_predicated(lo, gek, mid)
                nc.vector.copy_predicated(hi, gekn, mid)

            # =================== phase C: mask + recon ===================
            def phase_c(ck, lo, c0):
                r0 = ck * CHUNK
                mrm = []
                for rt in range(4):
                    col = ck * 4 + rt
                    m = mbuf.tile([128, CHUNK], fp16, tag="mrm")
                    nc.vector.tensor_scalar(
                        m, ebuf_g[col // GC][:, col % GC, :],
                        lo[:, col - c0:col - c0 + 1], None,
                        op0=Alu.is_ge)
                    mrm.append(m)
                masked = []
                for ft in range(4):
                    pm = ps_t.tile([128, CHUNK], fp16, tag="tr")
                    for rt in range(4):
                        nc.tensor.transpose(
                            pm[:, rt * 128:(rt + 1) * 128],
                            mrm[rt][:, ft * 128:(ft + 1) * 128], idf16)
                    mk = mbuf.tile([128, CHUNK], fp16, tag="mk")
                    nc.vector.tensor_mul(mk, pm, xh[:, ft, r0:r0 + CHUNK])
                    masked.append(mk)

                p = ps_h.tile([128, CHUNK], f32, tag="h")
                for ft in range(4):
                    nc.tensor.matmul(p, lhsT=wr1[:, ft, :], rhs=masked[ft],
                                     start=(ft == 0), stop=(ft == 3))
                rh = hbuf.tile([128, CHUNK], fp16, tag="rh")
                nc.scalar.activation(rh, p, Act.Relu, bias=br1)

                for (ot, o0, ow) in OSPLIT:
                    po = ps_w.tile([128, CHUNK], f32, tag="w")
                    nc.tensor.matmul(po[0:ow, :], lhsT=wr2[:, o0:o0 + ow],
                                     rhs=rh, start=True, stop=True)
                    of = obuf.tile([128, CHUNK], f32, tag="of")
                    nc.scalar.activation(of[0:ow, :], po[0:ow, :], Act.Identity,
                                         bias=br2[0:ow, ot, :])
                    nc.sync.dma_start(out_d[o0:o0 + ow, r0:r0 + CHUNK],
                                      of[0:ow, :])

            CPG = NCHUNK // NG
            for ck in range(NCHUNK):
                phase_a(ck)
            for g in range(NG):
                phase_b_init(g)
            for it in range(ITERS):
                for g in range(NG):
                    phase_b_iter(g, it)
            for g in range(NG):
                for ck in range(g * CPG, (g + 1) * CPG):
                    phase_c(ck, lo_g[g], g * GC)

    nc.compile()
    return nc


def kernel(**inputs):
    from concourse.bass_utils import run_bass_kernel_spmd

    x = np.asarray(inputs["x"], np.float32)
    names = ["W1", "b1", "W2", "b2", "W3", "b3", "Wg1", "bg1", "Wg2", "bg2",
             "Wr1", "br1", "Wr2", "br2"]
    W1, b1, W2, b2, W3, b3, Wg1, bg1, Wg2, bg2, Wr1, br1, Wr2, br2 = (
        np.asarray(inputs[n], np.float32) for n in names)

    A1, C1 = _calibrate(x, W1, b1, W2, b2, W3, b3, Wg1, bg1, Wg2, bg2)

    # weight prep (shared by all cores)
    def ksplit(Wm):  # [512,128] -> hi/lo [4,128,128] K tiles
        h, l = _split16(Wm)
        return (np.ascontiguousarray(h.reshape(4, 128, 128)),
                np.ascontiguousarray(l.reshape(4, 128, 128)))

    def msplit(Wm):  # [128,512] -> hi/lo [4,128,128] M tiles
        h, l = _split16(Wm)
        return (np.ascontiguousarray(h.reshape(128, 4, 128).transpose(1, 0, 2)),
                np.ascontiguousarray(l.reshape(128, 4, 128).transpose(1, 0, 2)))

    w1h, w1l = ksplit(W1)
    w2h, w2l = _split16(W2)
    w3h, w3l = msplit(W3)
    wg1h, wg1l = ksplit(Wg1)
    wg2h, wg2l = msplit(Wg2)
    wr1 = np.ascontiguousarray(
        np.asarray(Wr1, np.float16).reshape(4, 128, 128))
    wr2 = np.ascontiguousarray(np.asarray(Wr2, np.float16))
    coef = np.zeros((128, 2), np.float32)
    coef[:, 0] = A1 / 512.0
    coef[:, 1] = C1
    br2pad = np.zeros((3, 128, 1), np.float32)
    br2pad.reshape(-1)[:K] = br2
    ident = np.eye(128)
    shared = dict(
        w1h=w1h, w1l=w1l,
        w2h=np.ascontiguousarray(w2h), w2l=np.ascontiguousarray(w2l),
        w3h=w3h, w3l=w3l,
        wg1h=wg1h, wg1l=wg1l, wg2h=wg2h, wg2l=wg2l,
        wr1=wr1, wr2=wr2,
        b1=b1.reshape(128, 1), b2=b2.reshape(128, 1),
        b3=np.ascontiguousarray(b3.reshape(4, 128, 1)),
        bg1=bg1.reshape(128, 1),
        bg2=np.ascontiguousarray(bg2.reshape(4, 128, 1)),
        br1=br1.reshape(128, 1), br2=br2pad,
        coef=coef,
        idf32=ident.astype(np.float32),
        idf16=ident.astype(np.float16),
    )

    in_maps = []
    for i in range(NCORES):
        xs = x[i * R:(i + 1) * R]                 # [R, 512]
        xT = np.ascontiguousarray(xs.T)           # [512, R]
        xTh = np.asarray(xT, np.float16)
        xTl = np.asarray(xT - xTh.astype(np.float32), np.float16)
        m = dict(shared)
        m["xh"] = np.ascontiguousarray(xTh.reshape(4, 128, R))
        m["xl"] = np.ascontiguousarray(xTl.reshape(4, 128, R))
        in_maps.append(m)

    if "nc" not in _cache:
        _cache["nc"] = _build_program()
    nc = _cache["nc"]
    _cache["in_maps"] = in_maps

    res = run_bass_kernel_spmd(nc, in_maps, list(range(NCORES)))
    out = np.concatenate(
        [np.ascontiguousarray(res.results[i]["out"].T) for i in range(NCORES)],
        axis=0)
    return out.astype(np.float32)


if __name__ == "__main__":
    rng = np.random.default_rng(0)
    fake = {"x": rng.standard_normal((B, D), dtype=np.float32)}
    s = lambda f: 1.0 / np.sqrt(f)
    for nm, sh, fan in [("W1", (D, H), D), ("W2", (H, H), H), ("W3", (H, D), H),
                        ("Wg1", (D, H), D), ("Wg2", (H, D), H),
                        ("Wr1", (D, H), D), ("Wr2", (H, K), H)]:
        fake[nm] = rng.uniform(-s(fan), s(fan), sh).astype(np.float32)
    for nm, sh in [("b1", H), ("b2", H), ("b3", D), ("bg1", H), ("bg2", D),
                   ("br1", H), ("br2", K)]:
        fake[nm] = np.zeros(sh, np.float32)
    out = kernel(**fake)
    print("out", out.shape, out.dtype, float(np.abs(out).max()))


# revision 20
# speedup vs baseline: 1.1519x; 1.0050x over previous
"""Trainium2 Bass kernel for AdaptiveFeatureSelector (topk_masking).

Strategy (pure data parallel, 8 cores x 8192 rows):
 - All activations transposed on device: [features on partitions, rows on free].
 - Selector path (importance/gate nets -> combined) in fp16 hi/lo 3-term split
   matmuls (~f32 precision); reconstruction path in single fp16.
 - Exact per-row top-K=358 mask: PE-transpose combined to row-major, recenter
   by a host-calibrated affine-in-mean threshold estimate, then a 15-step
   binary search on fp16 residuals with per-partition tensor_scalar counts
   (accum_out). Mask built from the same fp16 residuals (consistent with the
   counts), PE-transposed back, applied to x, then recon MLP.
Host side: pre-transpose/split x, pre-split weights, calibrate the threshold
model on a 512-row sample; un-transpose the output. Device does all per-row
work.
"""

import sys

sys.path.insert(0, "/opt/trn_rl_repo")
import numpy as np

D = 512
H = 128
K = 358
B = 65536
NCORES = 8
R = B // NCORES          # rows per core
CHUNK = 512              # rows per chunk (one PSUM bank at f32)
NCHUNK = R // CHUNK      # 16
NCOL = R // 128          # 64 row-tiles per core
W_WIN = 0.025
ITERS = 14

_cache = {}


def _split16(a):
    hi = np.asarray(a, np.float16)
    lo = np.asarray(np.asarray(a, np.float32) - hi.astype(np.float32), np.float16)
    return hi, lo


def _sig(a):
    return 1.0 / (1.0 + np.exp(-a))


def _calibrate(x, W1, b1, W2, b2, W3, b3, Wg1, bg1, Wg2, bg2):
    xs = np.asarray(x[:512], np.float32)

    def mm3(a, Wm):
        ah, al = _split16(a)
        wh, wl = _split16(Wm)
        return (
            ah.astype(np.float32) @ wh.astype(np.float32)
            + ah.astype(np.float32) @ wl.astype(np.float32)
            + al.astype(np.float32) @ wh.astype(np.float32)
        )

    h = np.maximum(mm3(xs, W1) + b1, 0)
    h = np.maximum(mm3(h, W2) + b2, 0)
    imp = _sig(mm3(h, W3) + b3)
    g = np.maximum(mm3(xs, Wg1) + bg1, 0)
    gate = _sig(mm3(g, Wg2) + bg2)
    c = (imp * gate).astype(np.float32)
    mu = c.mean(1)
    thr = np.partition(c, D - K, axis=1)[:, D - K]
    A1, C1 = np.polyfit(mu, thr, 1)
    return float(A1), float(C1)


def _build_program():
    from concourse import bacc, mybir, tile

    f32 = mybir.dt.float32
    fp16 = mybir.dt.float16
    Act = mybir.ActivationFunctionType
    Alu = mybir.AluOpType

    nc = bacc.Bacc("TRN2", target_bir_lowering=False, debug=False,
                   num_devices=NCORES)

    def din(name, shape, dt=fp16):
        return nc.dram_tensor(name, shape, dt, kind="ExternalInput").ap()

    # DRAM layouts: weight K/M tile groups stored as [T, 128, 128] on host,
    # loaded into SBUF as [128, T, 128].
    xh_d = din("xh", [4, 128, R])
    xl_d = din("xl", [4, 128, R])
    w1h_d = din("w1h", [4, 128, 128]); w1l_d = din("w1l", [4, 128, 128])
    w2h_d = din("w2h", [128, 128]);    w2l_d = din("w2l", [128, 128])
    w3h_d = din("w3h", [4, 128, 128]); w3l_d = din("w3l", [4, 128, 128])
    wg1h_d = din("wg1h", [4, 128, 128]); wg1l_d = din("wg1l", [4, 128, 128])
    wg2h_d = din("wg2h", [4, 128, 128]); wg2l_d = din("wg2l", [4, 128, 128])
    wr1_d = din("wr1", [4, 128, 128])
    wr2_d = din("wr2", [128, K])
    b1_d = din("b1", [128, 1], f32)
    b2_d = din("b2", [128, 1], f32)
    b3_d = din("b3", [4, 128, 1], f32)
    bg1_d = din("bg1", [128, 1], f32)
    bg2_d = din("bg2", [4, 128, 1], f32)
    br1_d = din("br1", [128, 1], f32)
    br2_d = din("br2", [3, 128, 1], f32)   # padded 358 -> 384
    coef_d = din("coef", [128, 2], f32)
    idf32_d = din("idf32", [128, 128], f32)
    idf16_d = din("idf16", [128, 128], fp16)
    out_d = nc.dram_tensor("out", [K, R], f32, kind="ExternalOutput").ap()

    OSPLIT = [(0, 0, 128), (1, 128, 128), (2, 256, 102)]  # (tile, col0, width)

    with tile.TileContext(nc) as tc:
        with (
            tc.tile_pool(name="wts", bufs=1) as wts,
            tc.tile_pool(name="big", bufs=1) as big,
            tc.tile_pool(name="xls", bufs=4) as xls,
            tc.tile_pool(name="hbuf", bufs=3) as hbuf,
            tc.tile_pool(name="sgbuf", bufs=4) as sgbuf,
            tc.tile_pool(name="cbuf", bufs=4) as cbuf,
            tc.tile_pool(name="scr", bufs=2) as scr,
            tc.tile_pool(name="mbuf", bufs=4) as mbuf,
            tc.tile_pool(name="obuf", bufs=2) as obuf,
            tc.tile_pool(name="st", bufs=1) as st,
            tc.tile_pool(name="ps_h", bufs=2, space="PSUM") as ps_h,
            tc.tile_pool(name="ps_w", bufs=3, space="PSUM") as ps_w,
            tc.tile_pool(name="ps_t", bufs=2, space="PSUM") as ps_t,
        ):
            # ---- static loads (weights, biases, identities, coef) ----
            def ldt(dram, tiles, tag, dt=fp16):
                """[T,128,128] DRAM -> [128,T,128] SBUF."""
                t = wts.tile([128, tiles, 128], dt, tag=tag)
                nc.sync.dma_start(t, dram.rearrange("t p m -> p t m"))
                return t

            def ld2(dram, shape, tag, dt=f32):
                t = wts.tile(shape, dt, tag=tag)
                nc.sync.dma_start(t, dram)
                return t

            w1h = ldt(w1h_d, 4, "w1h"); w1l = ldt(w1l_d, 4, "w1l")
            w2h = ld2(w2h_d, [128, 128], "w2h", fp16)
            w2l = ld2(w2l_d, [128, 128], "w2l", fp16)
            w3h = ldt(w3h_d, 4, "w3h"); w3l = ldt(w3l_d, 4, "w3l")
            wg1h = ldt(wg1h_d, 4, "wg1h"); wg1l = ldt(wg1l_d, 4, "wg1l")
            wg2h = ldt(wg2h_d, 4, "wg2h"); wg2l = ldt(wg2l_d, 4, "wg2l")
            wr1 = ldt(wr1_d, 4, "wr1")
            wr2 = ld2(wr2_d, [128, K], "wr2", fp16)
            b1 = ld2(b1_d, [128, 1], "b1"); b2 = ld2(b2_d, [128, 1], "b2")
            b3 = wts.tile([128, 4, 1], f32, tag="b3")
            nc.sync.dma_start(b3, b3_d.rearrange("t p m -> p t m"))
            bg1 = ld2(bg1_d, [128, 1], "bg1")
            bg2 = wts.tile([128, 4, 1], f32, tag="bg2")
            nc.sync.dma_start(bg2, bg2_d.rearrange("t p m -> p t m"))
            br1 = ld2(br1_d, [128, 1], "br1")
            br2 = wts.tile([128, 3, 1], f32, tag="br2")
            nc.sync.dma_start(br2, br2_d.rearrange("t p m -> p t m"))
            coef = ld2(coef_d, [128, 2], "coef")
            idf32 = ld2(idf32_d, [128, 128], "idf32")
            idf16 = ld2(idf16_d, [128, 128], "idf16", fp16)

            # ---- persistent big buffers ----
            xh = big.tile([128, 4, R], fp16, tag="xh")      # resident x hi
            nc.sync.dma_start(xh, xh_d.rearrange("f p r -> p f r"))
            ebuf_g = []
            for g in range(4):
                tmp = big.tile([128, 16, 512], fp16, tag="ebuf%d" % g,
                               name="ebuf%d" % g)
                ebuf_g.append(tmp)
            mu = st.tile([128, NCOL], f32, tag="mu")
            t0 = st.tile([128, NCOL], f32, tag="t0")
            NG = 4
            GC = NCOL // NG
            lo_g, hi_g, mid_g, cnt_g, gek_g, gekn_g, nmid_g = [], [], [], [], [], [], []
            for g in range(NG):
                tmp = st.tile([128, GC], f32, tag="lo%d" % g, name="lo%d" % g)
                lo_g.append(tmp)
                tmp = st.tile([128, GC], f32, tag="hi%d" % g, name="hi%d" % g)
                hi_g.append(tmp)
                tmp = st.tile([128, GC], f32, tag="mid%d" % g, name="mid%d" % g)
                mid_g.append(tmp)
                tmp = st.tile([128, GC], f32, tag="cnt%d" % g, name="cnt%d" % g)
                cnt_g.append(tmp)
                tmp = st.tile([128, GC], mybir.dt.uint32, tag="gek%d" % g, name="gek%d" % g)
                gek_g.append(tmp)
                tmp = st.tile([128, GC], mybir.dt.uint32, tag="gekn%d" % g, name="gekn%d" % g)
                gekn_g.append(tmp)
                tmp = st.tile([128, GC], f32, tag="nmid%d" % g, name="nmid%d" % g)
                nmid_g.append(tmp)
            ones16 = st.tile([128, 512], fp16, tag="ones16")
            nc.vector.memset(ones16, 1.0)

            def split_evict(psum, bias):
                """relu(psum+bias) -> (hi, lo) fp16 pair."""
                hf = scr.tile([128, CHUNK], f32, tag="hf")
                nc.vector.tensor_scalar(hf, psum, bias, 0.0,
                                        op0=Alu.add, op1=Alu.max)
                th = hbuf.tile([128, CHUNK], fp16, tag="hh")
                nc.vector.tensor_copy(th, hf)
                tl = hbuf.tile([128, CHUNK], fp16, tag="hl")
                nc.vector.scalar_tensor_tensor(
                    tl, th, -1.0, hf, op0=Alu.mult, op1=Alu.add)
                return th, tl

            def net3(stats, movs, psum):
                """3-term split matmuls accumulated into psum.

                stats: list of (stat_hi, stat_lo) [128,128] stationary APs
                movs: list of (mov_hi, mov_lo) [128,CHUNK] moving APs
                """
                nk = len(stats)
                idx = 0
                total = nk * 3
                for ki in range(nk):
                    sh_, sl_ = stats[ki]
                    mh, ml = movs[ki]
                    for sta, mov in ((sh_, mh), (sh_, ml), (sl_, mh)):
                        nc.tensor.matmul(psum, lhsT=sta, rhs=mov,
                                         start=(idx == 0), stop=(idx == total - 1))
                        idx += 1

            # =================== phase A: selector ===================
            def phase_a(ck):
                r0 = ck * CHUNK
                xhc = [xh[:, ft, r0:r0 + CHUNK] for ft in range(4)]
                xlc = []
                for ft in range(4):
                    t = xls.tile([128, CHUNK], fp16, tag="xl")
                    nc.sync.dma_start(t, xl_d[ft, :, r0:r0 + CHUNK])
                    xlc.append(t)
                xmov = [(xhc[ki], xlc[ki]) for ki in range(4)]

                # importance net
                p = ps_h.tile([128, CHUNK], f32, tag="h")
                net3([(w1h[:, ki, :], w1l[:, ki, :]) for ki in range(4)],
                     xmov, p)
                h1h, h1l = split_evict(p, b1)

                p = ps_h.tile([128, CHUNK], f32, tag="h")
                net3([(w2h, w2l)], [(h1h, h1l)], p)
                h2h, h2l = split_evict(p, b2)

                sa = []
                for mt in range(4):
                    pw = ps_w.tile([128, CHUNK], f32, tag="w")
                    net3([(w3h[:, mt, :], w3l[:, mt, :])], [(h2h, h2l)], pw)
                    t = sgbuf.tile([128, CHUNK], f32, tag="sa")
                    nc.scalar.activation(t, pw, Act.Sigmoid, bias=b3[:, mt, :])
                    sa.append(t)

                # gate net
                p = ps_h.tile([128, CHUNK], f32, tag="h")
                net3([(wg1h[:, ki, :], wg1l[:, ki, :]) for ki in range(4)],
                     xmov, p)
                g1h, g1l = split_evict(p, bg1)

                ct = []
                for mt in range(4):
                    pw = ps_w.tile([128, CHUNK], f32, tag="w")
                    net3([(wg2h[:, mt, :], wg2l[:, mt, :])], [(g1h, g1l)], pw)
                    t = sgbuf.tile([128, CHUNK], f32, tag="sg")
                    nc.scalar.activation(t, pw, Act.Sigmoid, bias=bg2[:, mt, :])
                    c = cbuf.tile([128, CHUNK], f32, tag="c")
                    nc.vector.tensor_mul(c, sa[mt], t)
                    ct.append(c)

                # transpose combined to row-major, accumulate row sums,
                # recenter into fp16 residuals
                for rt in range(4):
                    col = ck * 4 + rt
                    ptr = ps_t.tile([128, CHUNK], f32, tag="tr")
                    for mt in range(4):
                        nc.tensor.transpose(
                            ptr[:, mt * 128:(mt + 1) * 128],
                            ct[mt][:, rt * 128:(rt + 1) * 128], idf32)
                    crm = scr.tile([128, CHUNK], f32, tag="crm")
                    nc.scalar.activation(crm, ptr, Act.Identity,
                                         accum_out=mu[:, col:col + 1])
                    nc.vector.tensor_scalar(
                        t0[:, col:col + 1], mu[:, col:col + 1],
                        coef[:, 0:1], coef[:, 1:2],
                        op0=Alu.mult, op1=Alu.add)
                    nc.vector.tensor_scalar(
                        ebuf_g[col // GC][:, col % GC, :], crm,
                        t0[:, col:col + 1], None,
                        op0=Alu.subtract)

            # =================== phase B: binary search (per group) ===================
            def phase_b_init(g):
                nc.vector.memset(lo_g[g], -W_WIN)
                nc.vector.memset(hi_g[g], W_WIN)

            def phase_b_iter(g, it):
                lo, hi, mid = lo_g[g], hi_g[g], mid_g[g]
                cnt, gek, gekn, negmid = cnt_g[g], gek_g[g], gekn_g[g], nmid_g[g]
                nc.vector.tensor_add(mid, lo, hi)
                nc.vector.tensor_scalar_mul(mid, mid, 0.5)
                if (it + g) % 2 == 0:
                    nc.vector.tensor_scalar_mul(negmid, mid, -1.0)
                    for i in range(GC):
                        sc = scr.tile([128, 512], fp16, tag="csc%d" % g)
                        nc.scalar.activation(
                            sc, ebuf_g[g][:, i, :], Act.Sign,
                            bias=negmid[:, i:i + 1],
                            accum_out=cnt[:, i:i + 1])
                    thrv = float(2 * K - 512)
                else:
                    for i in range(GC):
                        sc = scr.tile([128, 512], fp16, tag="csc%d" % g)
                        nc.vector.scalar_tensor_tensor(
                            sc, ebuf_g[g][:, i, :], mid[:, i:i + 1], ones16,
                            op0=Alu.is_ge, op1=Alu.mult,
                            accum_out=cnt[:, i:i + 1])
                    thrv = float(K)
                nc.vector.tensor_scalar(gek, cnt, thrv, None, op0=Alu.is_ge)
                nc.vector.tensor_scalar(gekn, cnt, thrv, None, op0=Alu.is_lt)
                nc.vector.copy_predicated(lo, gek, mid)
                nc.vector.copy_predicated(hi, gekn, mid)

            # =================== phase C: mask + recon ===================
            def phase_c(ck, lo, c0):
                r0 = ck * CHUNK
                mrm = []
                for rt in range(4):
                    col = ck * 4 + rt
                    m = mbuf.tile([128, CHUNK], fp16, tag="mrm")
                    nc.vector.tensor_scalar(
                        m, ebuf_g[col // GC][:, col % GC, :],
                        lo[:, col - c0:col - c0 + 1], None,
                        op0=Alu.is_ge)
                    mrm.append(m)
                masked = []
                for ft in range(4):
                    pm = ps_t.tile([128, CHUNK], fp16, tag="tr")
                    for rt in range(4):
                        nc.tensor.transpose(
                            pm[:, rt * 128:(rt + 1) * 128],
                            mrm[rt][:, ft * 128:(ft + 1) * 128], idf16)
                    mk = mbuf.tile([128, CHUNK], fp16, tag="mk")
                    nc.vector.tensor_mul(mk, pm, xh[:, ft, r0:r0 + CHUNK])
                    masked.append(mk)

                p = ps_h.tile([128, CHUNK], f32, tag="h")
                for ft in range(4):
                    nc.tensor.matmul(p, lhsT=wr1[:, ft, :], rhs=masked[ft],
                                     start=(ft == 0), stop=(ft == 3))
                rh = hbuf.tile([128, CHUNK], fp16, tag="rh")
                nc.scalar.activation(rh, p, Act.Relu, bias=br1)

                for (ot, o0, ow) in OSPLIT:
                    po = ps_w.tile([128, CHUNK], f32, tag="w")
                    nc.tensor.matmul(po[0:ow, :], lhsT=wr2[:, o0:o0 + ow],
                                     rhs=rh, start=True, stop=True)
                    of = obuf.tile([128, CHUNK], f32, tag="of")
                    nc.scalar.activation(of[0:ow, :], po[0:ow, :], Act.Identity,
                                         bias=br2[0:ow, ot, :])
                    nc.sync.dma_start(out_d[o0:o0 + ow, r0:r0 + CHUNK],
                                      of[0:ow, :])

            CPG = NCHUNK // NG
            for ck in range(NCHUNK):
                phase_a(ck)
            for g in range(NG):
                phase_b_init(g)
            for it in range(ITERS):
                for g in range(NG):
                    phase_b_iter(g, it)
            for g in range(NG):
                for ck in range(g * CPG, (g + 1) * CPG):
                    phase_c(ck, lo_g[g], g * GC)

    nc.compile()
    return nc


def kernel(**inputs):
    from concourse.bass_utils import run_bass_kernel_spmd

    x = np.asarray(inputs["x"], np.float32)
    names = ["W1", "b1", "W2", "b2", "W3", "b3", "Wg1", "bg1", "Wg2", "bg2",
             "Wr1", "br1", "Wr2", "br2"]
    W1, b1, W2, b2, W3, b3, Wg1, bg1, Wg2, bg2, Wr1, br1, Wr2, br2 = (
        np.asarray(inputs[n], np.float32) for n in names)

    A1, C1 = _calibrate(x, W1, b1, W2, b2, W3, b3, Wg1, bg1, Wg2, bg2)

    # weight prep (shared by all cores)
    def ksplit(Wm):  # [512,128] -> hi/lo [4,128,128] K tiles
        h, l = _split16(Wm)
        return (np.ascontiguousarray(h.reshape(4, 128, 128)),
                np.ascontiguousarray(l.reshape(4, 128, 128)))

    def msplit(Wm):  # [128,512] -> hi/lo [4,128,128] M tiles
        h, l = _split16(Wm)
        return (np.ascontiguousarray(h.reshape(128, 4, 128).transpose(1, 0, 2)),
                np.ascontiguousarray(l.reshape(128, 4, 128).transpose(1, 0, 2)))

    w1h, w1l = ksplit(W1)
    w2h, w2l = _split16(W2)
    w3h, w3l = msplit(W3)
    wg1h, wg1l = ksplit(Wg1)
    wg2h, wg2l = msplit(Wg2)
    wr1 = np.ascontiguousarray(
        np.asarray(Wr1, np.float16).reshape(4, 128, 128))
    wr2 = np.ascontiguousarray(np.asarray(Wr2, np.float16))
    coef = np.zeros((128, 2), np.float32)
    coef[:, 0] = A1 / 512.0
    coef[:, 1] = C1
    br2pad = np.zeros((3, 128, 1), np.float32)
    br2pad.reshape(-1)[:K] = br2
    ident = np.eye(128)
    shared = dict(
        w1h=w1h, w1l=w1l,
        w2h=np.ascontiguousarray(w2h), w2l=np.ascontiguousarray(w2l),
        w3h=w3h, w3l=w3l,
        wg1h=wg1h, wg1l=wg1l, wg2h=wg2h, wg2l=wg2l,
        wr1=wr1, wr2=wr2,
        b1=b1.reshape(128, 1), b2=b2.reshape(128, 1),
        b3=np.ascontiguousarray(b3.reshape(4, 128, 1)),
        bg1=bg1.reshape(128, 1),
        bg2=np.ascontiguousarray(bg2.reshape(4, 128, 1)),
        br1=br1.reshape(128, 1), br2=br2pad,
        coef=coef,
        idf32=ident.astype(np.float32),
        idf16=ident.astype(np.float16),
    )

    in_maps = []
    for i in range(NCORES):
        xs = x[i * R:(i + 1) * R]                 # [R, 512]
        xT = np.ascontiguousarray(xs.T)           # [512, R]
        xTh = np.asarray(xT, np.float16)
        xTl = np.asarray(xT - xTh.astype(np.float32), np.float16)
        m = dict(shared)
        m["xh"] = np.ascontiguousarray(xTh.reshape(4, 128, R))
        m["xl"] = np.ascontiguousarray(xTl.reshape(4, 128, R))
        in_maps.append(m)

    if "nc" not in _cache:
        _cache["nc"] = _build_program()
    nc = _cache["nc"]
    _cache["in_maps"] = in_maps

    res = run_bass_kernel_spmd(nc, in_maps, list(range(NCORES)))
    out = np.concatenate(
        [np.ascontiguousarray(res.results[i]["out"].T) for i in range(NCORES)],
        axis=0)
    return out.astype(np.float32)


if __name__ == "__main__":
    rng = np.random.default_rng(0)
    fake = {"x": rng.standard_normal((B, D), dtype=np.float32)}
    s = lambda f: 1.0 / np.sqrt(f)
    for nm, sh, fan in [("W1", (D, H), D), ("W2", (H, H), H), ("W3", (H, D), H),
                        ("Wg1", (D, H), D), ("Wg2", (H, D), H),
                        ("Wr1", (D, H), D), ("Wr2", (H, K), H)]:
        fake[nm] = rng.uniform(-s(fan), s(fan), sh).astype(np.float32)
    for nm, sh in [("b1", H), ("b2", H), ("b3", D), ("bg1", H), ("bg2", D),
                   ("br1", H), ("br2", K)]:
        fake[nm] = np.zeros(sh, np.float32)
    out = kernel(**fake)
    print("out", out.shape, out.dtype, float(np.abs(out).max()))
